# revision 1
# baseline (speedup 1.0000x reference)
"""Trainium2 Bass kernel for nn_CombinedLoss (rec + ident + attention-BCE).

Strategy
--------
The 256 MB correspondence_matrices BCE dominates (memory-bound).  Key identity:
gt_corr is nonzero only on the 5 diagonals |i-j|<=2 of each [N,N] matrix, so

    bce = w*(relu(x) - x*g + log1p(exp(-|x|)))   with w = 1+2g
        = softplus(x)                            where g == 0 (off-band, ~all)
        = softplus(x) + [2g*softplus(x) - (g+2g^2)*x]   on the band

Main stream (per core, 8 matrices = 32 MB): softplus via the ScalarE LUT as
ln(exp(x)+1) (two ACT passes; exact for |x| << 88, inputs are N(0,1)); the
second pass's accum_out produces the per-partition row sums for free.  Band
correction: strided DMA gathers the 5 diagonals (20 B/row) into compact
[128,40] tiles; two fused tensor_tensor_reduce ops per matrix apply
host-precomputed banded weights (pair visibility folded in on-device).

rec + ident losses are tiny (~3.5 MB) and sharded uniformly: each core takes
1/8 of the reconstruction points and 2 of the 16 (view,batch) identity pairs.
Each core writes a [128,24] f32 partial; the host only combines partials
(sums/mins/maxes over 8x128 lanes) into the final four scalars.
"""

import dataclasses
import os

import numpy as np

import concourse.bacc as bacc
import concourse.bass as bass
import concourse.mybir as mybir
from concourse.bass_utils import run_bass_kernel_spmd
from concourse.tile import TileContext

F32 = mybir.dt.float32
BF16 = mybir.dt.bfloat16
I32 = mybir.dt.int32
AF = mybir.ActivationFunctionType
OP = mybir.AluOpType
AX = mybir.AxisListType

N = 1024
V = 4
B = 4
F_FRAMES = 16
NCORES = 8
MAT_PER_CORE = 8          # V*V*B / 8
MATSZ = N * N             # elements per matrix
CORR_LEN = 2 + MAT_PER_CORE * MATSZ + 2

# rec shard: 1/8 of B*F*N = 65536 points -> 8192 points = [128, 64, 3]
REC_PTS = 8192

# final_acc column layout
C_ATT_MAIN = 0
C_ATT_C1 = 1
C_REC_SE = 2
C_REC_NUM = 3
C_REC_MN = 4     # 4..6
C_REC_MX = 7     # 7..9
C_ID_ERR = 10    # 10..13 (vb0x, vb0y, vb1x, vb1y)
C_ID_MN = 14     # 14..17
C_ID_MX = 18     # 18..21
C_ATT_C2 = 22
NCOLS = 24

_CACHE = {}
LAST_RESULTS = None


def _ap(t, offset, pairs):
    """Custom access pattern on a DRAM tensor handle."""
    return dataclasses.replace(t[:], ap=[list(p) for p in pairs], offset=offset)


def _build_program():
    parts = set(
        os.environ.get("KERNEL_PARTS", "main,setup,diag,rec,ident").split(",")
    )
    nc = bacc.Bacc("TRN2", target_bir_lowering=False, debug=False)

    corr = nc.dram_tensor("corrpad", [CORR_LEN], F32, kind="ExternalInput")
    recpred = nc.dram_tensor("recpred", [128, 192], F32, kind="ExternalInput")
    recgt = nc.dram_tensor("recgt", [128, 192], F32, kind="ExternalInput")
    recvis = nc.dram_tensor("recvis", [128, 64], I32, kind="ExternalInput")
    visr = nc.dram_tensor("visr", [128, 160], I32, kind="ExternalInput")
    visc = nc.dram_tensor("visc", [128, 160], I32, kind="ExternalInput")
    b1c = nc.dram_tensor("b1c", [128, 160], F32, kind="ExternalInput")
    b2c = nc.dram_tensor("b2c", [128, 160], F32, kind="ExternalInput")
    trk = nc.dram_tensor("trk", [2, 128, 256], F32, kind="ExternalInput")
    iprd = nc.dram_tensor("iprd", [2, 128, 384], F32, kind="ExternalInput")
    projbc = nc.dram_tensor("projbc", [128, 24], F32, kind="ExternalInput")
    out_d = nc.dram_tensor("out", [128, NCOLS], F32, kind="ExternalOutput")

    with TileContext(nc) as tc:
        with (
            tc.tile_pool(name="xpool", bufs=3) as xpool,
            tc.tile_pool(name="spool", bufs=1) as spool,
            tc.tile_pool(name="dpool", bufs=2) as dpool,
            tc.tile_pool(name="cpool", bufs=1) as cpool,
        ):
            # ---------------- constants / accumulators ----------------
            acc_main = cpool.tile([128, MAT_PER_CORE], F32, tag="acc_main")
            acc_c1 = cpool.tile([128, MAT_PER_CORE], F32, tag="acc_c1")
            acc_c2 = cpool.tile([128, MAT_PER_CORE], F32, tag="acc_c2")
            fin = cpool.tile([128, NCOLS], F32, tag="fin")
            nc.vector.memset(acc_main[:], 0.0)
            nc.vector.memset(acc_c1[:], 0.0)
            nc.vector.memset(acc_c2[:], 0.0)
            nc.vector.memset(fin[:], 0.0)

            setup_on = "setup" in parts
            if setup_on:
                vr_i = cpool.tile([128, 160], I32, tag="vr_i")
                vc_i = cpool.tile([128, 160], I32, tag="vc_i")
                g1 = cpool.tile([128, 160], F32, tag="g1")
                g2 = cpool.tile([128, 160], F32, tag="g2")
                b1t = cpool.tile([128, 160], F32, tag="b1t")
                b2t = cpool.tile([128, 160], F32, tag="b2t")
                nc.gpsimd.dma_start(vr_i[:], visr[:])
                nc.gpsimd.dma_start(vc_i[:], visc[:])
                nc.gpsimd.dma_start(b1t[:], b1c[:])
                nc.gpsimd.dma_start(b2t[:], b2c[:])
                vr_f = cpool.tile([128, 160], F32, tag="vr_f")
                vc_f = cpool.tile([128, 160], F32, tag="vc_f")
                nc.vector.tensor_copy(vr_f[:], vr_i[:])
                nc.vector.tensor_copy(vc_f[:], vc_i[:])
                # pair = vis[i] OR vis[j]  (values are 0/1)
                nc.vector.tensor_tensor(vr_f[:], vr_f[:], vc_f[:], OP.max)
                nc.vector.tensor_tensor(g1[:], b1t[:], vr_f[:], OP.mult)
                nc.vector.tensor_tensor(g2[:], b2t[:], vr_f[:], OP.mult)

            # ---------------- main BCE stream ----------------
            for m in range(MAT_PER_CORE):
                xt = xpool.tile([128, 8, 1024], F32, tag="xt")
                nc.sync.dma_start(
                    xt[:],
                    _ap(corr, 2 + m * MATSZ, [[1024, 128], [131072, 8], [1, 1024]]),
                )
                if "main" not in parts:
                    continue
                xf = xt[:].rearrange("p a b -> p (a b)")
                et = spool.tile([128, 8192], BF16, tag="et")
                nc.scalar.activation(et[:], xf, AF.Exp)
                lt = spool.tile([128, 8192], BF16, tag="lt")
                nc.scalar.activation(
                    lt[:], et[:], AF.Ln, bias=1.0,
                    accum_out=acc_main[:, m : m + 1],
                )

                # band correction on the 5 diagonals
                if not any(x in parts for x in ("diag", "diagdma", "diagact")):
                    continue
                bb = m % 4
                xd = dpool.tile([128, 40], F32, tag="xd")
                nc.sync.dma_start(
                    xd[:],
                    _ap(corr, m * MATSZ, [[8 * 1025, 128], [1025, 8], [1, 5]]),
                )
                if "diagdma" in parts and "diag" not in parts and "diagact" not in parts:
                    nc.vector.tensor_reduce(
                        acc_c1[:, m : m + 1], xd[:], axis=AX.X, op=OP.add
                    )
                    continue
                ed = dpool.tile([128, 40], F32, tag="ed")
                nc.scalar.activation(ed[:], xd[:], AF.Exp)
                ld = dpool.tile([128, 40], F32, tag="ld")
                nc.scalar.activation(ld[:], ed[:], AF.Ln, bias=1.0)
                if "diagact" in parts and "diag" not in parts:
                    nc.vector.tensor_reduce(
                        acc_c1[:, m : m + 1], ld[:], axis=AX.X, op=OP.add
                    )
                    continue
                sc1 = dpool.tile([128, 40], F32, tag="sc1")
                nc.vector.tensor_tensor(
                    sc1[:], g1[:, bb * 40 : bb * 40 + 40], ld[:], OP.mult
                )
                nc.vector.tensor_reduce(
                    acc_c1[:, m : m + 1], sc1[:], axis=AX.X, op=OP.add
                )
                sc2 = dpool.tile([128, 40], F32, tag="sc2")
                nc.vector.tensor_tensor(
                    sc2[:], g2[:, bb * 40 : bb * 40 + 40], xd[:], OP.mult
                )
                nc.vector.tensor_reduce(
                    acc_c2[:, m : m + 1], sc2[:], axis=AX.X, op=OP.add
                )

            # ---------------- reconstruction loss partials ----------------
            if "rec" in parts:
                prt = cpool.tile([128, 192], F32, tag="prt")
                grt = cpool.tile([128, 192], F32, tag="grt")
                vrt = cpool.tile([128, 64], I32, tag="vrt")
                nc.gpsimd.dma_start(prt[:], recpred[:])
                nc.gpsimd.dma_start(grt[:], recgt[:])
                nc.gpsimd.dma_start(vrt[:], recvis[:])
                mf = cpool.tile([128, 64], F32, tag="mf")
                nc.vector.tensor_copy(mf[:], vrt[:])
                dd = cpool.tile([128, 192], F32, tag="dd")
                nc.vector.tensor_tensor(dd[:], prt[:], grt[:], OP.subtract)
                d2 = cpool.tile([128, 192], F32, tag="d2")
                nc.vector.tensor_tensor(d2[:], dd[:], dd[:], OP.mult)
                se3 = cpool.tile([128, 64], F32, tag="se3")
                nc.vector.tensor_reduce(
                    se3[:], d2[:].rearrange("p (q c) -> p q c", c=3), axis=AX.X, op=OP.add
                )
                se3m = cpool.tile([128, 64], F32, tag="se3m")
                nc.vector.tensor_tensor(se3m[:], se3[:], mf[:], OP.mult)
                nc.vector.tensor_reduce(
                    fin[:, C_REC_SE : C_REC_SE + 1], se3m[:], axis=AX.X, op=OP.add
                )
                nc.vector.tensor_reduce(
                    fin[:, C_REC_NUM : C_REC_NUM + 1], mf[:], axis=AX.X, op=OP.add
                )
                # masked min / max of gt per coordinate
                bmn = cpool.tile([128, 192], F32, tag="bmn")
                bmx = cpool.tile([128, 192], F32, tag="bmx")
                nc.vector.memset(bmn[:], 1e30)
                nc.vector.memset(bmx[:], -1e30)
                for c in range(3):
                    nc.vector.copy_predicated(
                        bmn[:, c : 192 : 3], vrt[:], grt[:, c : 192 : 3]
                    )
                    nc.vector.copy_predicated(
                        bmx[:, c : 192 : 3], vrt[:], grt[:, c : 192 : 3]
                    )
                nc.vector.tensor_reduce(
                    fin[:, C_REC_MN : C_REC_MN + 3],
                    bmn[:].rearrange("p (q c) -> p c q", c=3), axis=AX.X, op=OP.min,
                )
                nc.vector.tensor_reduce(
                    fin[:, C_REC_MX : C_REC_MX + 3],
                    bmx[:].rearrange("p (q c) -> p c q", c=3), axis=AX.X, op=OP.max,
                )

            # ---------------- identity loss partials (2 vb slots) ----------------
            if "ident" not in parts:
                pass
            else:
                psb = cpool.tile([128, 24], F32, tag="psb")
                nc.gpsimd.dma_start(psb[:], projbc[:])

                for i in range(2):
                    tk = cpool.tile([128, 256], F32, tag=f"tk{i}")
                    pd = cpool.tile([128, 384], F32, tag=f"pd{i}")
                    nc.gpsimd.dma_start(tk[:], trk[i])
                    nc.gpsimd.dma_start(pd[:], iprd[i])
                    Xc = pd[:, 0:384:3]
                    Yc = pd[:, 1:384:3]
                    Zc = pd[:, 2:384:3]

                    def cS(col):
                        return psb[:, col : col + 1]

                    base = i * 12

                    def lincomb(row, tag):
                        # P[row,0]*x + P[row,1]*y + P[row,2]*z + P[row,3]
                        t0 = cpool.tile([128, 128], F32, tag=f"{tag}0_{i}")
                        t1 = cpool.tile([128, 128], F32, tag=f"{tag}1_{i}")
                        nc.vector.tensor_scalar(
                            t0[:], Xc, cS(base + row * 4 + 0), cS(base + row * 4 + 3),
                            OP.mult, OP.add,
                        )
                        nc.vector.tensor_scalar(
                            t1[:], Yc, cS(base + row * 4 + 1), None, OP.mult
                        )
                        nc.vector.tensor_tensor(t0[:], t0[:], t1[:], OP.add)
                        nc.vector.tensor_scalar(
                            t1[:], Zc, cS(base + row * 4 + 2), None, OP.mult
                        )
                        nc.vector.tensor_tensor(t0[:], t0[:], t1[:], OP.add)
                        return t0

                    den = lincomb(2, "den")
                    nc.vector.tensor_scalar_add(den[:], den[:], 1e-10)
                    rd = cpool.tile([128, 128], F32, tag=f"rd{i}")
                    nc.vector.reciprocal(rd[:], den[:])
                    nx = lincomb(0, "nx")
                    ny = lincomb(1, "ny")
                    nc.vector.tensor_tensor(nx[:], nx[:], rd[:], OP.mult)
                    nc.vector.tensor_tensor(ny[:], ny[:], rd[:], OP.mult)
                    nc.vector.tensor_tensor(nx[:], nx[:], tk[:, 0:256:2], OP.subtract)
                    nc.vector.tensor_tensor(ny[:], ny[:], tk[:, 1:256:2], OP.subtract)
                    sqx = cpool.tile([128, 128], F32, tag=f"sqx{i}")
                    nc.vector.tensor_tensor(sqx[:], nx[:], nx[:], OP.mult)
                    nc.vector.tensor_reduce(
                        fin[:, C_ID_ERR + 2 * i : C_ID_ERR + 2 * i + 1],
                        sqx[:], axis=AX.X, op=OP.add,
                    )
                    sqy = cpool.tile([128, 128], F32, tag=f"sqy{i}")
                    nc.vector.tensor_tensor(sqy[:], ny[:], ny[:], OP.mult)
                    nc.vector.tensor_reduce(
                        fin[:, C_ID_ERR + 2 * i + 1 : C_ID_ERR + 2 * i + 2],
                        sqy[:], axis=AX.X, op=OP.add,
                    )
                    tkv = tk[:].rearrange("p (q c) -> p c q", c=2)
                    nc.vector.tensor_reduce(
                        fin[:, C_ID_MN + 2 * i : C_ID_MN + 2 * i + 2],
                        tkv, axis=AX.X, op=OP.min,
                    )
                    nc.vector.tensor_reduce(
                        fin[:, C_ID_MX + 2 * i : C_ID_MX + 2 * i + 2],
                        tkv, axis=AX.X, op=OP.max,
                    )

            # ---------------- final reductions + store ----------------
            nc.vector.tensor_reduce(
                fin[:, C_ATT_MAIN : C_ATT_MAIN + 1], acc_main[:], axis=AX.X, op=OP.add
            )
            nc.vector.tensor_reduce(
                fin[:, C_ATT_C1 : C_ATT_C1 + 1], acc_c1[:], axis=AX.X, op=OP.add
            )
            nc.vector.tensor_reduce(
                fin[:, C_ATT_C2 : C_ATT_C2 + 1], acc_c2[:], axis=AX.X, op=OP.add
            )
            nc.sync.dma_start(out_d[:], fin[:])

    nc.compile()
    return nc


def _host_constants():
    """Banded weights + index tables (data independent)."""
    i_idx = np.arange(128)[:, None] * 8 + np.arange(8)[None, :]        # [128,8]
    d_off = np.arange(5) - 2
    ipd = i_idx[:, :, None] + d_off[None, None, :]                     # [128,8,5]
    valid = (ipd >= 0) & (ipd < N)
    beta = np.array([0.49, 0.7, 1.0, 0.7, 0.49], np.float64)
    b1 = np.where(valid, (2.0 * beta)[None, None, :], 0.0)
    b2 = np.where(valid, (-(beta + 2.0 * beta**2))[None, None, :], 0.0)
    b1 = np.tile(b1.reshape(128, 40), (1, 4)).astype(np.float32)
    b2 = np.tile(b2.reshape(128, 40), (1, 4)).astype(np.float32)
    return i_idx, ipd, valid, b1, b2


def kernel(refined_points, gt_points, visibility, projection_matrices,
           tracks_2d, correspondence_matrices):
    global LAST_RESULTS
    refined_points = np.ascontiguousarray(refined_points, np.float32)
    gt_points = np.ascontiguousarray(gt_points, np.float32)
    visibility = np.ascontiguousarray(visibility, np.int32)
    projection_matrices = np.ascontiguousarray(projection_matrices, np.float32)
    tracks_2d = np.ascontiguousarray(tracks_2d, np.float32)
    corr = np.ascontiguousarray(correspondence_matrices, np.float32)

    if "nc" not in _CACHE:
        _CACHE["nc"] = _build_program()
    nc = _CACHE["nc"]

    i_idx, ipd, valid, b1, b2 = _host_constants()
    vis0 = visibility[:, 0, :]                                         # [4,1024]
    visr = np.repeat(vis0[:, i_idx][:, :, :, None], 5, axis=3)         # [4,128,8,5]
    visr = visr.reshape(4, 128, 40).transpose(1, 0, 2).reshape(128, 160)
    visc = np.where(valid[None], vis0[:, np.clip(ipd, 0, N - 1)], 0)   # [4,128,8,5]
    visc = visc.reshape(4, 128, 40).transpose(1, 0, 2).reshape(128, 160)
    visr = np.ascontiguousarray(visr, np.int32)
    visc = np.ascontiguousarray(visc, np.int32)

    corr64 = corr.reshape(V * V * B, MATSZ)
    pred_flat = refined_points.reshape(B * F_FRAMES * N, 3)
    gt_flat = gt_points.reshape(B * F_FRAMES * N, 3)
    vis_flat = visibility.reshape(B * F_FRAMES * N)
    pvals = projection_matrices.reshape(V * B, 12)

    in_maps = []
    for c in range(NCORES):
        cp = np.zeros(CORR_LEN, np.float32)
        cp[2:-2] = corr64[c * MAT_PER_CORE : (c + 1) * MAT_PER_CORE].ravel()
        rp = pred_flat[c * REC_PTS : (c + 1) * REC_PTS].reshape(128, 192)
        rg = gt_flat[c * REC_PTS : (c + 1) * REC_PTS].reshape(128, 192)
        rv = vis_flat[c * REC_PTS : (c + 1) * REC_PTS].reshape(128, 64)
        vbs = [2 * c, 2 * c + 1]
        tks = np.stack([tracks_2d[vb // 4, vb % 4].reshape(128, 256) for vb in vbs])
        ipr = np.stack(
            [refined_points[vb % 4].reshape(128, 384) for vb in vbs]
        )
        pb = np.broadcast_to(
            np.concatenate([pvals[vb] for vb in vbs])[None, :], (128, 24)
        )
        in_maps.append({
            "corrpad": cp,
            "recpred": np.ascontiguousarray(rp),
            "recgt": np.ascontiguousarray(rg),
            "recvis": np.ascontiguousarray(rv),
            "visr": visr,
            "visc": visc,
            "b1c": b1,
            "b2c": b2,
            "trk": np.ascontiguousarray(tks, np.float32),
            "iprd": np.ascontiguousarray(ipr, np.float32),
            "projbc": np.ascontiguousarray(pb, np.float32),
        })

    trace = bool(int(os.environ.get("KERNEL_TRACE", "0")))
    ncr = int(os.environ.get("KERNEL_NCORES", str(NCORES)))
    res = run_bass_kernel_spmd(
        nc, in_maps[:ncr], core_ids=list(range(ncr)), trace=trace,
        stitch_traces=trace,
    )
    LAST_RESULTS = res
    P = np.stack([r["out"] for r in res.results]).astype(np.float64)   # [8,128,24]

    # ---- attention ----
    att_sum = (
        P[:, :, C_ATT_MAIN].sum() + P[:, :, C_ATT_C1].sum() + P[:, :, C_ATT_C2].sum()
    )
    att = att_sum / (V * V * B * N * N)

    # ---- reconstruction ----
    se = P[:, :, C_REC_SE].sum()
    num = 3.0 * P[:, :, C_REC_NUM].sum()
    mn = P[:, :, C_REC_MN : C_REC_MN + 3].min(axis=(0, 1))
    mx = P[:, :, C_REC_MX : C_REC_MX + 3].max(axis=(0, 1))
    scale = (mx - mn).max() + 1e-6
    if not num > 0:
        scale = 1.0
    rec = (se / max(num, 1.0)) / scale**2

    # ---- identity ----
    vls = []
    for vb in range(V * B):
        c, i = vb // 2, vb % 2
        ex = P[c, :, C_ID_ERR + 2 * i]
        ey = P[c, :, C_ID_ERR + 2 * i + 1]
        mnx = P[c, :, C_ID_MN + 2 * i]
        mny = P[c, :, C_ID_MN + 2 * i + 1]
        mxx = P[c, :, C_ID_MX + 2 * i]
        mxy = P[c, :, C_ID_MX + 2 * i + 1]
        for f in range(F_FRAMES):
            s = slice(8 * f, 8 * f + 8)
            whx = max(224.0, mxx[s].max() - mnx[s].min() + 1e-6)
            why = max(224.0, mxy[s].max() - mny[s].min() + 1e-6)
            vls.append((ex[s].sum() / whx**2 + ey[s].sum() / why**2) / N)
    ident = float(np.mean(vls))

    total = 1.0 * rec + 1.0 * ident + 0.5 * att
    return (
        np.float32(total), np.float32(rec), np.float32(ident), np.float32(att),
    )



# revision 2
# speedup vs baseline: 1.3395x; 1.3395x over previous
"""Trainium2 Bass kernel for nn_CombinedLoss (rec + ident + attention-BCE).

Strategy
--------
The 256 MB correspondence_matrices BCE dominates (memory-bound).  gt_corr is
nonzero only on the 5 diagonals |i-j|<=2, so

    bce = softplus(x)                                  off-band (~all elements)
        = softplus(x) + [2g*softplus(x) - (g+2g^2)*x]  on the band

Main stream (per core, 8 matrices = 32 MB), designed so ScalarE does ONE
activation pass instead of two (the old Exp->Ln per-matrix pattern also
thrashed ACT table loads, 16 per kernel):

    sum softplus(x) = sum_blocks ln( prod_block (1 + e^x) )

Per half-matrix tile: ACT Exp -> bf16, then a DVE halving cascade of
contiguous tensor_tensor multiplies (2x rate in bf16) folds 32 elements into
one product; a single Ln with accum_out at the end touches only 1/32 of the
elements and needs exactly one table switch.

The DMA uses a skewed access pattern (row stride 1025 instead of 1024) so the
banded diagonals land in fixed columns 0..4 of every partition row -- the band
is then a free SBUF slice (no strided 20 B/row gather DMAs).  The skew makes
tiles overrun matrix boundaries by 1024 elements; the host subtracts the
softplus of those double-counted boundary cells (7*1024 + pads per core).

rec + ident losses are tiny (~3.5 MB) and sharded uniformly: each core takes
1/8 of the reconstruction points and 2 of the 16 (view,batch) identity pairs.
Each core writes a [128,24] f32 partial; the host combines partials into the
final four scalars.
"""

import dataclasses
import os

import numpy as np

import concourse.bacc as bacc
import concourse.bass as bass
import concourse.mybir as mybir
from concourse.bass_utils import run_bass_kernel_spmd
from concourse.tile import TileContext

F32 = mybir.dt.float32
BF16 = mybir.dt.bfloat16
I32 = mybir.dt.int32
AF = mybir.ActivationFunctionType
OP = mybir.AluOpType
AX = mybir.AxisListType

N = 1024
V = 4
B = 4
F_FRAMES = 16
NCORES = 8
MAT_PER_CORE = 8          # V*V*B / 8
MATSZ = N * N             # elements per matrix
SLOT = N + 1              # skewed row stride: diag band at cols 0..4
NHALF = 16                # half-matrix tiles per core
HALF_SLOTS = 4            # 4 skewed rows of SLOT per partition per tile
HFREE = HALF_SLOTS * SLOT  # 4100 elements per partition per tile
HALF_SPAN = 128 * HFREE // 1  # flat elements spanned by one half tile
END_PAD = 1024 - 2        # skew overrun past the last matrix
CORR_LEN = 2 + MAT_PER_CORE * MATSZ + END_PAD
PADV = -100.0             # softplus(PADV) == 0 in f32/bf16
LNB = 132                 # ln-buffer cols per half tile (128 po32 + 4 rest)
BANDC = NHALF * 20        # band staging cols (4 slots x 5 diags per tile)

# rec shard: 1/8 of B*F*N = 65536 points -> 8192 points = [128, 64, 3]
REC_PTS = 8192

# final_acc column layout
C_ATT_MAIN = 0
C_ATT_C1 = 1
C_REC_SE = 2
C_REC_NUM = 3
C_REC_MN = 4     # 4..6
C_REC_MX = 7     # 7..9
C_ID_ERR = 10    # 10..13 (vb0x, vb0y, vb1x, vb1y)
C_ID_MN = 14     # 14..17
C_ID_MX = 18     # 18..21
C_ATT_C2 = 22
NCOLS = 24

_CACHE = {}
LAST_RESULTS = None


def _ap(t, offset, pairs):
    """Custom access pattern on a DRAM tensor handle."""
    return dataclasses.replace(t[:], ap=[list(p) for p in pairs], offset=offset)


def _build_program():
    nc = bacc.Bacc("TRN2", target_bir_lowering=False, debug=False)

    corr = nc.dram_tensor("corrpad", [CORR_LEN], F32, kind="ExternalInput")
    recpred = nc.dram_tensor("recpred", [128, 192], F32, kind="ExternalInput")
    recgt = nc.dram_tensor("recgt", [128, 192], F32, kind="ExternalInput")
    recvis = nc.dram_tensor("recvis", [128, 64], I32, kind="ExternalInput")
    g1c = nc.dram_tensor("g1c", [128, BANDC], F32, kind="ExternalInput")
    g2c = nc.dram_tensor("g2c", [128, BANDC], F32, kind="ExternalInput")
    trk = nc.dram_tensor("trk", [2, 128, 256], F32, kind="ExternalInput")
    iprd = nc.dram_tensor("iprd", [2, 128, 384], F32, kind="ExternalInput")
    projbc = nc.dram_tensor("projbc", [128, 24], F32, kind="ExternalInput")
    out_d = nc.dram_tensor("out", [128, NCOLS], F32, kind="ExternalOutput")

    with TileContext(nc) as tc:
        with (
            tc.tile_pool(name="xpool", bufs=4) as xpool,
            tc.tile_pool(name="epool", bufs=2) as epool,
            tc.tile_pool(name="hpool", bufs=2) as hpool,
            tc.tile_pool(name="cpool", bufs=1) as cpool,
        ):
            fin = cpool.tile([128, NCOLS], F32, tag="fin")
            nc.vector.memset(fin[:], 0.0)
            xb = cpool.tile([128, BANDC], F32, tag="xb")
            lnbuf = cpool.tile([128, NHALF * LNB], BF16, tag="lnbuf")
            g1t = cpool.tile([128, BANDC], F32, tag="g1t")
            g2t = cpool.tile([128, BANDC], F32, tag="g2t")
            nc.gpsimd.dma_start(g1t[:], g1c[:])
            nc.gpsimd.dma_start(g2t[:], g2c[:])

            # ---------------- reconstruction loss partials ----------------
            prt = cpool.tile([128, 192], F32, tag="prt")
            grt = cpool.tile([128, 192], F32, tag="grt")
            vrt = cpool.tile([128, 64], I32, tag="vrt")
            nc.gpsimd.dma_start(prt[:], recpred[:])
            nc.gpsimd.dma_start(grt[:], recgt[:])
            nc.gpsimd.dma_start(vrt[:], recvis[:])
            mf = cpool.tile([128, 64], F32, tag="mf")
            nc.vector.tensor_copy(mf[:], vrt[:])
            dd = cpool.tile([128, 192], F32, tag="dd")
            nc.vector.tensor_tensor(dd[:], prt[:], grt[:], OP.subtract)
            d2 = cpool.tile([128, 192], F32, tag="d2")
            nc.vector.tensor_tensor(d2[:], dd[:], dd[:], OP.mult)
            se3 = cpool.tile([128, 64], F32, tag="se3")
            nc.vector.tensor_reduce(
                se3[:], d2[:].rearrange("p (q c) -> p q c", c=3), axis=AX.X, op=OP.add
            )
            se3m = cpool.tile([128, 64], F32, tag="se3m")
            nc.vector.tensor_tensor(se3m[:], se3[:], mf[:], OP.mult)
            nc.vector.tensor_reduce(
                fin[:, C_REC_SE : C_REC_SE + 1], se3m[:], axis=AX.X, op=OP.add
            )
            nc.vector.tensor_reduce(
                fin[:, C_REC_NUM : C_REC_NUM + 1], mf[:], axis=AX.X, op=OP.add
            )
            # masked min / max of gt per coordinate
            bmn = cpool.tile([128, 192], F32, tag="bmn")
            bmx = cpool.tile([128, 192], F32, tag="bmx")
            nc.vector.memset(bmn[:], 1e30)
            nc.vector.memset(bmx[:], -1e30)
            for c in range(3):
                nc.vector.copy_predicated(
                    bmn[:, c : 192 : 3], vrt[:], grt[:, c : 192 : 3]
                )
                nc.vector.copy_predicated(
                    bmx[:, c : 192 : 3], vrt[:], grt[:, c : 192 : 3]
                )
            nc.vector.tensor_reduce(
                fin[:, C_REC_MN : C_REC_MN + 3],
                bmn[:].rearrange("p (q c) -> p c q", c=3), axis=AX.X, op=OP.min,
            )
            nc.vector.tensor_reduce(
                fin[:, C_REC_MX : C_REC_MX + 3],
                bmx[:].rearrange("p (q c) -> p c q", c=3), axis=AX.X, op=OP.max,
            )

            # ---------------- identity loss partials (2 vb slots) ----------------
            psb = cpool.tile([128, 24], F32, tag="psb")
            nc.gpsimd.dma_start(psb[:], projbc[:])

            for i in range(2):
                tk = cpool.tile([128, 256], F32, tag=f"tk{i}")
                pd = cpool.tile([128, 384], F32, tag=f"pd{i}")
                nc.gpsimd.dma_start(tk[:], trk[i])
                nc.gpsimd.dma_start(pd[:], iprd[i])
                Xc = pd[:, 0:384:3]
                Yc = pd[:, 1:384:3]
                Zc = pd[:, 2:384:3]

                def cS(col):
                    return psb[:, col : col + 1]

                base = i * 12

                def lincomb(row, tag):
                    # P[row,0]*x + P[row,1]*y + P[row,2]*z + P[row,3]
                    t0 = cpool.tile([128, 128], F32, tag=f"{tag}0_{i}")
                    t1 = cpool.tile([128, 128], F32, tag=f"{tag}1_{i}")
                    nc.vector.tensor_scalar(
                        t0[:], Xc, cS(base + row * 4 + 0), cS(base + row * 4 + 3),
                        OP.mult, OP.add,
                    )
                    nc.vector.tensor_scalar(
                        t1[:], Yc, cS(base + row * 4 + 1), None, OP.mult
                    )
                    nc.vector.tensor_tensor(t0[:], t0[:], t1[:], OP.add)
                    nc.vector.tensor_scalar(
                        t1[:], Zc, cS(base + row * 4 + 2), None, OP.mult
                    )
                    nc.vector.tensor_tensor(t0[:], t0[:], t1[:], OP.add)
                    return t0

                den = lincomb(2, "den")
                nc.vector.tensor_scalar_add(den[:], den[:], 1e-10)
                rd = cpool.tile([128, 128], F32, tag=f"rd{i}")
                nc.vector.reciprocal(rd[:], den[:])
                nx = lincomb(0, "nx")
                ny = lincomb(1, "ny")
                nc.vector.tensor_tensor(nx[:], nx[:], rd[:], OP.mult)
                nc.vector.tensor_tensor(ny[:], ny[:], rd[:], OP.mult)
                nc.vector.tensor_tensor(nx[:], nx[:], tk[:, 0:256:2], OP.subtract)
                nc.vector.tensor_tensor(ny[:], ny[:], tk[:, 1:256:2], OP.subtract)
                sqx = cpool.tile([128, 128], F32, tag=f"sqx{i}")
                nc.vector.tensor_tensor(sqx[:], nx[:], nx[:], OP.mult)
                nc.vector.tensor_reduce(
                    fin[:, C_ID_ERR + 2 * i : C_ID_ERR + 2 * i + 1],
                    sqx[:], axis=AX.X, op=OP.add,
                )
                sqy = cpool.tile([128, 128], F32, tag=f"sqy{i}")
                nc.vector.tensor_tensor(sqy[:], ny[:], ny[:], OP.mult)
                nc.vector.tensor_reduce(
                    fin[:, C_ID_ERR + 2 * i + 1 : C_ID_ERR + 2 * i + 2],
                    sqy[:], axis=AX.X, op=OP.add,
                )
                tkv = tk[:].rearrange("p (q c) -> p c q", c=2)
                nc.vector.tensor_reduce(
                    fin[:, C_ID_MN + 2 * i : C_ID_MN + 2 * i + 2],
                    tkv, axis=AX.X, op=OP.min,
                )
                nc.vector.tensor_reduce(
                    fin[:, C_ID_MX + 2 * i : C_ID_MX + 2 * i + 2],
                    tkv, axis=AX.X, op=OP.max,
                )

            # ---------------- main BCE stream: 16 skewed half-tiles ----------------
            for h in range(NHALF):
                m, hh = h // 2, h % 2
                off = m * MATSZ + hh * (HALF_SLOTS * 128 * SLOT)
                xt = xpool.tile([128, HALF_SLOTS, SLOT], F32, tag="xt")
                nc.sync.dma_start(
                    xt[:],
                    _ap(corr, off, [[SLOT, 128], [128 * SLOT, HALF_SLOTS], [1, SLOT]]),
                )
                # stage the diag band (cols 0..4 of each skewed row slot)
                xbv = xb[:, h * 20 : (h + 1) * 20].rearrange(
                    "p (a k) -> p a k", k=5
                )
                nc.vector.tensor_copy(xbv, xt[:, :, 0:5])
                et = epool.tile([128, HFREE], BF16, tag="et")
                nc.scalar.activation(et[:], xt[:].rearrange("p a b -> p (a b)"), AF.Exp)
                # (1+e) halving cascade -> products of 32 in bf16
                u = hpool.tile([128, 2048], BF16, tag="u")
                nc.vector.tensor_scalar_add(u[:], et[:, 0:2048], 1.0)
                h1 = hpool.tile([128, 2048], BF16, tag="h1")
                nc.vector.scalar_tensor_tensor(
                    h1[:], et[:, 2048:4096], 1.0, u[:], OP.add, OP.mult
                )
                h2 = hpool.tile([128, 1024], BF16, tag="h2")
                nc.vector.tensor_tensor(h2[:], h1[:, 0:1024], h1[:, 1024:2048], OP.mult)
                h3 = hpool.tile([128, 512], BF16, tag="h3")
                nc.vector.tensor_tensor(h3[:], h2[:, 0:512], h2[:, 512:1024], OP.mult)
                h4 = hpool.tile([128, 256], BF16, tag="h4")
                nc.vector.tensor_tensor(h4[:], h3[:, 0:256], h3[:, 256:512], OP.mult)
                nc.vector.tensor_tensor(
                    lnbuf[:, h * LNB : h * LNB + 128],
                    h4[:, 0:128], h4[:, 128:256], OP.mult,
                )
                nc.vector.tensor_scalar_add(
                    lnbuf[:, h * LNB + 128 : (h + 1) * LNB], et[:, 4096:4100], 1.0
                )

            # ---------------- attention epilogue ----------------
            eb = cpool.tile([128, BANDC], BF16, tag="eb")
            nc.scalar.activation(eb[:], xb[:], AF.Exp)
            junk = cpool.tile([128, NHALF * LNB], BF16, tag="junk")
            nc.scalar.activation(
                junk[:], lnbuf[:], AF.Ln,
                accum_out=fin[:, C_ATT_MAIN : C_ATT_MAIN + 1],
            )
            spb = cpool.tile([128, BANDC], F32, tag="spb")
            nc.scalar.activation(spb[:], eb[:], AF.Ln, bias=1.0)
            t1 = cpool.tile([128, BANDC], F32, tag="t1")
            nc.vector.tensor_tensor(t1[:], g1t[:], spb[:], OP.mult)
            nc.vector.tensor_reduce(
                fin[:, C_ATT_C1 : C_ATT_C1 + 1], t1[:], axis=AX.X, op=OP.add
            )
            t2 = cpool.tile([128, BANDC], F32, tag="t2")
            nc.vector.tensor_tensor(t2[:], g2t[:], xb[:], OP.mult)
            nc.vector.tensor_reduce(
                fin[:, C_ATT_C2 : C_ATT_C2 + 1], t2[:], axis=AX.X, op=OP.add
            )
            nc.sync.dma_start(out_d[:], fin[:])

    nc.compile()
    return nc


def _band_tables(vis0, core):
    """Host band weights g1/g2 [128, BANDC] for one core, matching the skewed
    band layout: col = h*20 + a*5 + k, row r = p + 128*(4*(h%2)+a)."""
    p = np.arange(128)
    beta = np.array([0.49, 0.7, 1.0, 0.7, 0.49], np.float64)
    g1 = np.zeros((128, BANDC), np.float64)
    g2 = np.zeros((128, BANDC), np.float64)
    for h in range(NHALF):
        m, hh = h // 2, h % 2
        b = (core * MAT_PER_CORE + m) % B
        for a in range(HALF_SLOTS):
            r = p + 128 * (4 * hh + a)                       # [128]
            for k in range(5):
                c = r + k - 2
                valid = (c >= 0) & (c < N)
                cc = np.clip(c, 0, N - 1)
                pair = np.logical_or(vis0[b, r] > 0, vis0[b, cc] > 0)
                g = beta[k] * pair * valid
                g1[:, h * 20 + a * 5 + k] = 2.0 * g
                g2[:, h * 20 + a * 5 + k] = -(g + 2.0 * g * g)
    return g1.astype(np.float32), g2.astype(np.float32)


def kernel(refined_points, gt_points, visibility, projection_matrices,
           tracks_2d, correspondence_matrices):
    global LAST_RESULTS
    refined_points = np.ascontiguousarray(refined_points, np.float32)
    gt_points = np.ascontiguousarray(gt_points, np.float32)
    visibility = np.ascontiguousarray(visibility, np.int32)
    projection_matrices = np.ascontiguousarray(projection_matrices, np.float32)
    tracks_2d = np.ascontiguousarray(tracks_2d, np.float32)
    corr = np.ascontiguousarray(correspondence_matrices, np.float32)

    if "nc" not in _CACHE:
        _CACHE["nc"] = _build_program()
    nc = _CACHE["nc"]

    vis0 = visibility[:, 0, :]                                         # [4,1024]
    corr64 = corr.reshape(V * V * B, MATSZ)
    pred_flat = refined_points.reshape(B * F_FRAMES * N, 3)
    gt_flat = gt_points.reshape(B * F_FRAMES * N, 3)
    vis_flat = visibility.reshape(B * F_FRAMES * N)
    pvals = projection_matrices.reshape(V * B, 12)

    in_maps = []
    corrections = np.zeros(NCORES, np.float64)
    for c in range(NCORES):
        cp = np.full(CORR_LEN, PADV, np.float32)
        cp[2 : 2 + MAT_PER_CORE * MATSZ] = corr64[
            c * MAT_PER_CORE : (c + 1) * MAT_PER_CORE
        ].ravel()
        # skew overrun: boundary cells read twice (plus pads read once)
        sp = lambda v: np.logaddexp(0.0, v.astype(np.float64)).sum()
        corrections[c] = sp(cp[0:2]) + sp(cp[8 * MATSZ + 2 : 8 * MATSZ + 1024])
        for m in range(1, MAT_PER_CORE):
            corrections[c] += sp(cp[m * MATSZ : m * MATSZ + 1024])
        g1, g2 = _band_tables(vis0, c)
        rp = pred_flat[c * REC_PTS : (c + 1) * REC_PTS].reshape(128, 192)
        rg = gt_flat[c * REC_PTS : (c + 1) * REC_PTS].reshape(128, 192)
        rv = vis_flat[c * REC_PTS : (c + 1) * REC_PTS].reshape(128, 64)
        vbs = [2 * c, 2 * c + 1]
        tks = np.stack([tracks_2d[vb // 4, vb % 4].reshape(128, 256) for vb in vbs])
        ipr = np.stack(
            [refined_points[vb % 4].reshape(128, 384) for vb in vbs]
        )
        pb = np.broadcast_to(
            np.concatenate([pvals[vb] for vb in vbs])[None, :], (128, 24)
        )
        in_maps.append({
            "corrpad": cp,
            "recpred": np.ascontiguousarray(rp),
            "recgt": np.ascontiguousarray(rg),
            "recvis": np.ascontiguousarray(rv),
            "g1c": g1,
            "g2c": g2,
            "trk": np.ascontiguousarray(tks, np.float32),
            "iprd": np.ascontiguousarray(ipr, np.float32),
            "projbc": np.ascontiguousarray(pb, np.float32),
        })

    trace = bool(int(os.environ.get("KERNEL_TRACE", "0")))
    ncr = int(os.environ.get("KERNEL_NCORES", str(NCORES)))
    res = run_bass_kernel_spmd(
        nc, in_maps[:ncr], core_ids=list(range(ncr)), trace=trace,
        stitch_traces=False,
    )
    LAST_RESULTS = res
    P = np.stack([r["out"] for r in res.results]).astype(np.float64)   # [8,128,24]

    # ---- attention ----
    att_sum = (
        P[:, :, C_ATT_MAIN].sum() + P[:, :, C_ATT_C1].sum() + P[:, :, C_ATT_C2].sum()
        - corrections[:ncr].sum()
    )
    att = att_sum / (V * V * B * N * N)

    # ---- reconstruction ----
    se = P[:, :, C_REC_SE].sum()
    num = 3.0 * P[:, :, C_REC_NUM].sum()
    mn = P[:, :, C_REC_MN : C_REC_MN + 3].min(axis=(0, 1))
    mx = P[:, :, C_REC_MX : C_REC_MX + 3].max(axis=(0, 1))
    scale = (mx - mn).max() + 1e-6
    if not num > 0:
        scale = 1.0
    rec = (se / max(num, 1.0)) / scale**2

    # ---- identity ----
    vls = []
    for vb in range(V * B):
        c, i = vb // 2, vb % 2
        ex = P[c, :, C_ID_ERR + 2 * i]
        ey = P[c, :, C_ID_ERR + 2 * i + 1]
        mnx = P[c, :, C_ID_MN + 2 * i]
        mny = P[c, :, C_ID_MN + 2 * i + 1]
        mxx = P[c, :, C_ID_MX + 2 * i]
        mxy = P[c, :, C_ID_MX + 2 * i + 1]
        for f in range(F_FRAMES):
            s = slice(8 * f, 8 * f + 8)
            whx = max(224.0, mxx[s].max() - mnx[s].min() + 1e-6)
            why = max(224.0, mxy[s].max() - mny[s].min() + 1e-6)
            vls.append((ex[s].sum() / whx**2 + ey[s].sum() / why**2) / N)
    ident = float(np.mean(vls))

    total = 1.0 * rec + 1.0 * ident + 0.5 * att
    return (
        np.float32(total), np.float32(rec), np.float32(ident), np.float32(att),
    )
